# revision 17
# baseline (speedup 1.0000x reference)
"""Dark channel prior loss on 8 trn2 NeuronCores.

Reference computes: reflect-pad H/W by 7, min over (C, H, W) per image,
mean over batch. Reflect padding only duplicates interior values, so it
cannot change a min — the loss is exactly mean_b(min_chw(x[b])).

Data-parallel: 4 images (12 MiB) per core, viewed as 16 chunks of
[128, 1536] f32. Raw bacc kernel (no TileContext): GpSimd issues all 16
chunk DMAs back-to-back (hoisted before the init barrier so the HBM
stream starts at launch; the barrier's Pool DRAIN is defused to a NOP
carrying the same semaphore protocol, since a real GpSimd DRAIN waits
for all outstanding SWDGE DMAs). VectorE min-reduces each chunk to one
column of a [128, 16] partial as its DMA completes (cumulative sem
waits; SWDGE completions are FIFO). The partial is DMA'd out; the host
finishes min-over-partitions/chunks and the batch mean (tiny).

Measured ~46 µs/core: ~6.5 µs runtime launch protocol + 12.58 MB HBM
stream at ~420 GB/s (~31 µs) + one exposed chunk reduce + out-DMA.
"""

import numpy as np

import concourse.bass as bass  # noqa: F401
from concourse import bacc, mybir
from concourse.bass_utils import run_bass_kernel_spmd

N_CORES = 8
B = 32
PER_CORE = B // N_CORES  # 4 images per core
P = 128
F = 3 * 512 * 512 // P  # 6144 elements per partition per image
CHUNKS_PER_IMG = 2
NCHUNK = PER_CORE * CHUNKS_PER_IMG
FC = F // CHUNKS_PER_IMG

_nc_cache = None


def _build_nc():
    nc = bacc.Bacc(trn_type="TRN2", debug=False, num_devices=N_CORES)
    x = nc.dram_tensor("x", [NCHUNK, P, FC], mybir.dt.float32, kind="ExternalInput")
    out = nc.dram_tensor("out", [P, NCHUNK], mybir.dt.float32, kind="ExternalOutput")
    x_ap = x.ap()
    out_ap = out.ap()

    dma_sem = nc.alloc_semaphore("dma_done")
    red_sem = nc.alloc_semaphore("red_done")
    out_sem = nc.alloc_semaphore("out_done")
    buf = nc.alloc_sbuf_tensor("buf", [P, NCHUNK * FC], mybir.dt.float32)
    partial = nc.alloc_sbuf_tensor("partial", [P, NCHUNK], mybir.dt.float32)

    load_insts = []
    for c in range(NCHUNK):
        bi = nc.gpsimd.dma_start(
            buf.ap()[:, c * FC : (c + 1) * FC], x_ap[c]
        ).then_inc(dma_sem, 16)
        load_insts.append(bi.ins)
    for c in range(NCHUNK):
        red = nc.vector.tensor_reduce(
            out=partial.ap()[:, c : c + 1],
            in_=buf.ap()[:, c * FC : (c + 1) * FC],
            axis=mybir.AxisListType.X,
            op=mybir.AluOpType.min,
        )._wait_ge(dma_sem, 16 * (c + 1))
        if c == NCHUNK - 1:
            red.then_inc(red_sem)
    out_bi = nc.gpsimd.dma_start(out_ap[:], partial.ap())._wait_ge(
        red_sem, 1
    ).then_inc(out_sem, 16)
    nc.gpsimd.wait_ge(out_sem, 16)
    # Reset kernel sems so a repeat execution of the same NEFF starts clean.
    for s in (dma_sem, red_sem, out_sem):
        nc.gpsimd.sem_clear(s)

    # Hoist the load DMAs to right after GpSimd's register preamble (same
    # splice point bacc uses for its kernel-barrier collective). Nothing
    # before the init barrier reads buf, and dma_sem was reset by the
    # previous execution's tail.
    entry = nc.main_func.blocks[0]
    assert nc.gpsimd.preamble_end is not None
    for inst in load_insts:
        entry.instructions.remove(inst)
    idx = entry.instructions.index(nc.gpsimd.preamble_end) + 1
    entry.instructions[idx:idx] = load_insts

    # GpSimd's DRAIN waits for ALL outstanding SWDGE DMAs, so the init
    # barrier's Pool drain would serialize the hoisted stream. Replace Pool
    # drains before the out-DMA with NOPs carrying the same semaphore
    # protocol — every data dependency rides an explicit sem.
    pool = nc.gpsimd.engine
    for pos, inst in enumerate(list(entry.instructions)):
        if inst is out_bi.ins:
            break
        if isinstance(inst, mybir.InstDrain) and inst.engine == pool:
            nop = mybir.InstNoOp(name=nc.get_next_instruction_name(), ins=[], outs=[])
            nop.engine = pool
            nop.sync_info = inst.sync_info
            nc.register_instruction(nop)
            entry.instructions[pos] = nop

    nc.finalize()
    return nc


def _run_spmd(x: np.ndarray, **kwargs):
    """x: full [32,3,512,512] f32. Returns BassKernelResults."""
    global _nc_cache
    if _nc_cache is None:
        _nc_cache = _build_nc()
    shards = np.ascontiguousarray(x).reshape(N_CORES, NCHUNK, P, FC)
    in_maps = [{"x": shards[i]} for i in range(N_CORES)]
    return run_bass_kernel_spmd(
        _nc_cache, in_maps, core_ids=list(range(N_CORES)), **kwargs
    )


def kernel(input_image: np.ndarray) -> np.ndarray:
    x = np.asarray(input_image, dtype=np.float32)
    res = _run_spmd(x)
    # [8, 128, NCHUNK] -> per-image mins -> mean over 32 images
    partials = np.stack([r["out"] for r in res.results])
    per_image = partials.reshape(N_CORES, P, PER_CORE, CHUNKS_PER_IMG).min(axis=(1, 3))
    return np.asarray(per_image.mean(), dtype=np.float32)


# revision 19
# speedup vs baseline: 1.0149x; 1.0149x over previous
"""Dark channel prior loss on 8 trn2 NeuronCores.

Reference computes: reflect-pad H/W by 7, min over (C, H, W) per image,
mean over batch. Reflect padding only duplicates interior values, so it
cannot change a min — the loss is exactly mean_b(min_chw(x[b])).

Data-parallel: 4 images (12 MiB) per core, viewed as 16 chunks of
[128, 1536] f32. Raw bacc kernel (no TileContext): GpSimd issues all 16
chunk DMAs back-to-back (hoisted before the init barrier so the HBM
stream starts at launch; the barrier's Pool DRAIN is defused to a NOP
carrying the same semaphore protocol, since a real GpSimd DRAIN waits
for all outstanding SWDGE DMAs). VectorE min-reduces each chunk to one
column of a [128, 16] partial as its DMA completes (cumulative sem
waits; SWDGE completions are FIFO). The partial is DMA'd out; the host
finishes min-over-partitions/chunks and the batch mean (tiny).

Measured ~46 µs/core: ~6.5 µs runtime launch protocol + 12.58 MB HBM
stream at ~420 GB/s (~31 µs) + one exposed chunk reduce + out-DMA.
"""

import numpy as np

import concourse.bass as bass  # noqa: F401
from concourse import bacc, mybir
from concourse.bass_utils import run_bass_kernel_spmd


def _install_ntff_hook():
    """This image's antenv lacks axon_hooks, so a traced run (trace=True or
    BASS_TRACE=1) would crash inside run_bass_kernel_spmd on the import.
    Synthesize the module around trn_boot's ctypes NTFF hook; degrade
    silently if any piece is missing."""
    import sys
    import types

    if "antenv.axon_hooks" in sys.modules:
        return
    try:
        sys.path.insert(0, "/root/.axon_site")
        from trn_agent_boot.trn_boot import _ntff_profile_via_ctypes

        hook = _ntff_profile_via_ctypes("/opt/axon/libaxon_pjrt.so")
        mod = types.ModuleType("antenv.axon_hooks")
        mod._hook = hook
        mod.get_axon_ntff_profile_hook = lambda: mod._hook
        mod.set_axon_ntff_profile_hook = lambda h: setattr(mod, "_hook", h)
        sys.modules["antenv.axon_hooks"] = mod
    except Exception:
        pass


_install_ntff_hook()

N_CORES = 8
B = 32
PER_CORE = B // N_CORES  # 4 images per core
P = 128
F = 3 * 512 * 512 // P  # 6144 elements per partition per image
CHUNKS_PER_IMG = 4
NCHUNK = PER_CORE * CHUNKS_PER_IMG
FC = F // CHUNKS_PER_IMG

_nc_cache = None


def _build_nc():
    nc = bacc.Bacc(trn_type="TRN2", debug=False, num_devices=N_CORES)
    x = nc.dram_tensor("x", [NCHUNK, P, FC], mybir.dt.float32, kind="ExternalInput")
    out = nc.dram_tensor("out", [P, NCHUNK], mybir.dt.float32, kind="ExternalOutput")
    x_ap = x.ap()
    out_ap = out.ap()

    dma_sem = nc.alloc_semaphore("dma_done")
    red_sem = nc.alloc_semaphore("red_done")
    out_sem = nc.alloc_semaphore("out_done")
    buf = nc.alloc_sbuf_tensor("buf", [P, NCHUNK * FC], mybir.dt.float32)
    partial = nc.alloc_sbuf_tensor("partial", [P, NCHUNK], mybir.dt.float32)

    load_insts = []
    for c in range(NCHUNK):
        bi = nc.gpsimd.dma_start(
            buf.ap()[:, c * FC : (c + 1) * FC], x_ap[c]
        ).then_inc(dma_sem, 16)
        load_insts.append(bi.ins)
    for c in range(NCHUNK):
        red = nc.vector.tensor_reduce(
            out=partial.ap()[:, c : c + 1],
            in_=buf.ap()[:, c * FC : (c + 1) * FC],
            axis=mybir.AxisListType.X,
            op=mybir.AluOpType.min,
        )._wait_ge(dma_sem, 16 * (c + 1))
        if c == NCHUNK - 1:
            red.then_inc(red_sem)
    out_bi = nc.gpsimd.dma_start(out_ap[:], partial.ap())._wait_ge(
        red_sem, 1
    ).then_inc(out_sem, 16)
    nc.gpsimd.wait_ge(out_sem, 16)
    # Reset kernel sems so a repeat execution of the same NEFF starts clean.
    for s in (dma_sem, red_sem, out_sem):
        nc.gpsimd.sem_clear(s)

    # Hoist the load DMAs to right after GpSimd's register preamble (same
    # splice point bacc uses for its kernel-barrier collective). Nothing
    # before the init barrier reads buf, and dma_sem was reset by the
    # previous execution's tail.
    entry = nc.main_func.blocks[0]
    assert nc.gpsimd.preamble_end is not None
    for inst in load_insts:
        entry.instructions.remove(inst)
    idx = entry.instructions.index(nc.gpsimd.preamble_end) + 1
    entry.instructions[idx:idx] = load_insts

    # GpSimd's DRAIN waits for ALL outstanding SWDGE DMAs, so the init
    # barrier's Pool drain would serialize the hoisted stream. Replace Pool
    # drains before the out-DMA with NOPs carrying the same semaphore
    # protocol — every data dependency rides an explicit sem.
    pool = nc.gpsimd.engine
    for pos, inst in enumerate(list(entry.instructions)):
        if inst is out_bi.ins:
            break
        if isinstance(inst, mybir.InstDrain) and inst.engine == pool:
            nop = mybir.InstNoOp(name=nc.get_next_instruction_name(), ins=[], outs=[])
            nop.engine = pool
            nop.sync_info = inst.sync_info
            nc.register_instruction(nop)
            entry.instructions[pos] = nop

    nc.finalize()
    return nc


def _run_spmd(x: np.ndarray, **kwargs):
    """x: full [32,3,512,512] f32. Returns BassKernelResults."""
    global _nc_cache
    if _nc_cache is None:
        _nc_cache = _build_nc()
    shards = np.ascontiguousarray(x).reshape(N_CORES, NCHUNK, P, FC)
    in_maps = [{"x": shards[i]} for i in range(N_CORES)]
    return run_bass_kernel_spmd(
        _nc_cache, in_maps, core_ids=list(range(N_CORES)), **kwargs
    )


def kernel(input_image: np.ndarray) -> np.ndarray:
    x = np.asarray(input_image, dtype=np.float32)
    res = _run_spmd(x)
    # [8, 128, NCHUNK] -> per-image mins -> mean over 32 images
    partials = np.stack([r["out"] for r in res.results])
    per_image = partials.reshape(N_CORES, P, PER_CORE, CHUNKS_PER_IMG).min(axis=(1, 3))
    return np.asarray(per_image.mean(), dtype=np.float32)


# revision 22
# speedup vs baseline: 1.0717x; 1.0560x over previous
"""Dark channel prior loss on 8 trn2 NeuronCores.

Reference computes: reflect-pad H/W by 7, min over (C, H, W) per image,
mean over batch. Reflect padding only duplicates interior values, so it
cannot change a min — the loss is exactly mean_b(min_chw(x[b])).

Data-parallel: 4 images (12 MiB) per core, viewed as 16 chunks of
[128, 1536] f32. Raw bacc kernel (no TileContext): GpSimd issues all 16
chunk DMAs back-to-back (hoisted before the init barrier so the HBM
stream starts at launch; the barrier's Pool DRAIN is defused to a NOP
carrying the same semaphore protocol, since a real GpSimd DRAIN waits
for all outstanding SWDGE DMAs). VectorE min-reduces each chunk to one
column of a [128, 16] partial as its DMA completes (cumulative sem
waits; SWDGE completions are FIFO). The partial is DMA'd out; the host
finishes min-over-partitions/chunks and the batch mean (tiny).

Measured ~46 µs/core: ~6.5 µs runtime launch protocol + 12.58 MB HBM
stream at ~420 GB/s (~31 µs) + one exposed chunk reduce + out-DMA.
"""

import numpy as np

import concourse.bass as bass  # noqa: F401
from concourse import bacc, mybir
from concourse.bass_utils import run_bass_kernel_spmd


def _install_ntff_hook():
    """This image's antenv lacks axon_hooks, so a traced run (trace=True or
    BASS_TRACE=1) would crash inside run_bass_kernel_spmd on the import.
    Synthesize the module around trn_boot's ctypes NTFF hook; degrade
    silently if any piece is missing."""
    import sys
    import types

    if "antenv.axon_hooks" in sys.modules:
        return
    try:
        sys.path.insert(0, "/root/.axon_site")
        from trn_agent_boot.trn_boot import _ntff_profile_via_ctypes

        hook = _ntff_profile_via_ctypes("/opt/axon/libaxon_pjrt.so")
        mod = types.ModuleType("antenv.axon_hooks")
        mod._hook = hook
        mod.get_axon_ntff_profile_hook = lambda: mod._hook
        mod.set_axon_ntff_profile_hook = lambda h: setattr(mod, "_hook", h)
        sys.modules["antenv.axon_hooks"] = mod
    except Exception:
        pass


_install_ntff_hook()

N_CORES = 8
B = 32
PER_CORE = B // N_CORES  # 4 images per core
P = 128
F = 3 * 512 * 512 // P  # 6144 elements per partition per image
CHUNKS_PER_IMG = 4
NCHUNK = PER_CORE * CHUNKS_PER_IMG
FC = F // CHUNKS_PER_IMG

_nc_cache = None


def _build_nc(optimize: bool = True):
    nc = bacc.Bacc(trn_type="TRN2", debug=False, num_devices=N_CORES)
    x = nc.dram_tensor("x", [NCHUNK, P, FC], mybir.dt.float32, kind="ExternalInput")
    out = nc.dram_tensor("out", [P, NCHUNK], mybir.dt.float32, kind="ExternalOutput")
    x_ap = x.ap()
    out_ap = out.ap()

    dma_sem = nc.alloc_semaphore("dma_done")
    red_sem = nc.alloc_semaphore("red_done")
    out_sem = nc.alloc_semaphore("out_done")
    buf = nc.alloc_sbuf_tensor("buf", [P, NCHUNK * FC], mybir.dt.float32)
    partial = nc.alloc_sbuf_tensor("partial", [P, NCHUNK], mybir.dt.float32)

    load_insts = []
    for c in range(NCHUNK):
        bi = nc.gpsimd.dma_start(
            buf.ap()[:, c * FC : (c + 1) * FC], x_ap[c]
        ).then_inc(dma_sem, 16)
        load_insts.append(bi.ins)
    for c in range(NCHUNK):
        red = nc.vector.tensor_reduce(
            out=partial.ap()[:, c : c + 1],
            in_=buf.ap()[:, c * FC : (c + 1) * FC],
            axis=mybir.AxisListType.X,
            op=mybir.AluOpType.min,
        )._wait_ge(dma_sem, 16 * (c + 1))
        if c == NCHUNK - 1:
            red.then_inc(red_sem)
    out_bi = nc.gpsimd.dma_start(out_ap[:], partial.ap())._wait_ge(
        red_sem, 1
    ).then_inc(out_sem, 16)
    nc.gpsimd.wait_ge(out_sem, 16)
    # Reset kernel sems so a repeat execution of the same NEFF starts clean.
    for s in (dma_sem, red_sem, out_sem):
        nc.gpsimd.sem_clear(s)

    if optimize:
        # Hoist the load DMAs to right after GpSimd's register preamble
        # (same splice point bacc uses for its kernel-barrier collective)
        # so the HBM stream starts before the init barrier. Nothing before
        # the barrier reads buf, and dma_sem was reset by the previous
        # execution's tail. Then defuse the init barrier's Pool DRAINs:
        # a GpSimd DRAIN waits for ALL outstanding SWDGE DMAs, which
        # would serialize the hoisted stream; a NOP carrying the same
        # semaphore protocol preserves the barrier — every data
        # dependency rides an explicit sem. Applied to a scratch list so
        # a failure leaves the (still-correct, ~3us slower) unhoisted
        # layout intact.
        try:
            entry = nc.main_func.blocks[0]
            insts = list(entry.instructions)
            assert nc.gpsimd.preamble_end is not None
            for inst in load_insts:
                insts.remove(inst)
            idx = insts.index(nc.gpsimd.preamble_end) + 1
            insts[idx:idx] = load_insts

            pool = nc.gpsimd.engine
            for pos, inst in enumerate(insts):
                if inst is out_bi.ins:
                    break
                if isinstance(inst, mybir.InstDrain) and inst.engine == pool:
                    nop = mybir.InstNoOp(
                        name=nc.get_next_instruction_name(), ins=[], outs=[]
                    )
                    nop.engine = pool
                    nop.sync_info = inst.sync_info
                    nc.register_instruction(nop)
                    insts[pos] = nop

            entry.instructions[:] = insts
        except Exception:
            return _build_nc(optimize=False)

    nc.finalize()
    return nc


def _run_spmd(x: np.ndarray, **kwargs):
    """x: full [32,3,512,512] f32. Returns BassKernelResults."""
    global _nc_cache
    if _nc_cache is None:
        _nc_cache = _build_nc()
    shards = np.ascontiguousarray(x).reshape(N_CORES, NCHUNK, P, FC)
    in_maps = [{"x": shards[i]} for i in range(N_CORES)]
    return run_bass_kernel_spmd(
        _nc_cache, in_maps, core_ids=list(range(N_CORES)), **kwargs
    )


def kernel(input_image: np.ndarray) -> np.ndarray:
    x = np.asarray(input_image, dtype=np.float32)
    res = _run_spmd(x)
    # [8, 128, NCHUNK] -> per-image mins -> mean over 32 images
    partials = np.stack([r["out"] for r in res.results])
    per_image = partials.reshape(N_CORES, P, PER_CORE, CHUNKS_PER_IMG).min(axis=(1, 3))
    return np.asarray(per_image.mean(), dtype=np.float32)
